# revision 1
# baseline (speedup 1.0000x reference)
"""Causal self-attention (B=4, T=2048, C=1024, H=16) on 8 TRN2 NeuronCores.

Sharding: core = (batch b, head-group hg). Data parallel over B (4), tensor
parallel over heads (2 groups of 8). Each core computes a partial output
projection for its 8 heads; the host sums the two partials per batch
(row-parallel linear unshard).

Per-core pipeline (all matmuls fp32r, accumulate fp32 in PSUM):
  0) PE-transpose x [T,C] -> xT [C,T] (contraction for qkv is over C)
  1) kT = wk^T xT ([512, T], head h at partition rows (h%2)*64..),
     qT same but stored zero-PADDED per head: qT_pad[:, h, :] has head h's
     64 dims in its partition half and zeros in the other half, so the
     scores matmul can run with K=128 (full PE rows; the pad half multiplies
     the other head's kT rows by zero). K=64 matmuls starve the PE HAM
     activity monitor and the clock gates down to 1.2 GHz.
     v = x wv ([T, 512]) stored per (head, t-tile) with a ones column
     appended -> v_aug [128k, 65]
  2) per head, per 512-wide q block: scoresT [128k, 512q] = kT_blk^T @ qT_blk
     (transposed layout so the softmax denominator comes from the PE via the
     ones column of v_aug instead of a cross-partition reduce),
     p = exp(scoresT/32) (no max subtraction: |scores| <= ~2.1), exp batched
     2 blocks per ACTIVATE over a [128,1024] 2-bank psum window to amortize
     the 352-cycle ACT fixed overhead,
     causal: skip blocks above the diagonal, trim + triangular-mask the 4
     diagonal blocks,
     yT_aug [65, 512q] += v_aug^T @ p  (row 64 = softmax denominators),
     yT = yT_aug[0:64] * (1/denominator broadcast across partitions)
  3) out_partial [T, 1024] = yT_all^T @ wp, accumulated over 4 k-tiles
"""
import numpy as np
from contextlib import ExitStack

import concourse.bass as bass
import concourse.mybir as mybir
import concourse.tile as tile
from concourse import bacc
from concourse.bass_utils import run_bass_kernel_spmd
from concourse.masks import make_identity

F32 = mybir.dt.float32
F32R = mybir.dt.float32r
AF = mybir.ActivationFunctionType
F16 = mybir.dt.float16

T = 2048
C = 1024
H_PER_CORE = 8          # heads per core
D = 64                  # head dim
GC = H_PER_CORE * D     # 512 channels per head-group
SCALE = 1.0 / 32.0      # C ** -0.5
N_CORES = 8


def build(nc):
    x_d = nc.dram_tensor("x", [T, C], F32R, kind="ExternalInput").ap()
    wq_d = nc.dram_tensor("wq", [C, GC], F32R, kind="ExternalInput").ap()
    wk_d = nc.dram_tensor("wk", [C, GC], F32R, kind="ExternalInput").ap()
    wv_d = nc.dram_tensor("wv", [C, GC], F32R, kind="ExternalInput").ap()
    wp_d = nc.dram_tensor("wp", [GC, C], F32R, kind="ExternalInput").ap()
    out_d = nc.dram_tensor("out", [T, C], F32, kind="ExternalOutput").ap()

    NT = T // 128        # 16 t-tiles
    NC_ = C // 128       # 8 c-tiles
    NQSB = T // 512      # 4 q superblocks

    with tile.TileContext(nc) as tc, ExitStack() as ctx:
        const = ctx.enter_context(tc.tile_pool(name="const", bufs=1))
        persist = ctx.enter_context(tc.tile_pool(name="persist", bufs=1))

        ident_f32 = const.tile([128, 128], F32)
        make_identity(nc, ident_f32[:])
        ident = const.tile([128, 128], F32R)
        nc.vector.tensor_copy(ident[:], ident_f32[:])
        # tri_mask[k, j] = 1.0 if k <= j else 0.0
        tri_mask = const.tile([128, 128], F16)
        nc.gpsimd.memset(tri_mask[:], 1.0)
        nc.gpsimd.affine_select(
            out=tri_mask[:], in_=tri_mask[:],
            compare_op=mybir.AluOpType.is_ge, fill=0.0, base=0,
            pattern=[[1, 128]], channel_multiplier=-1,
        )

        # persistent activations
        qT_pad = persist.tile([128, H_PER_CORE, T], F32R)  # [head, t], zero-padded
        kT_sb = persist.tile([128, 4, T], F32R)            # [m-tile, t]
        v_aug = persist.tile([128, H_PER_CORE, NT, 65], F16)
        yT_sb = persist.tile([128, 4, T], F32R)
        nc.gpsimd.memset(qT_pad[:].bitcast(F32), 0.0)
        nc.gpsimd.memset(v_aug[:, :, :, 64], 1.0)

        copy_engines = [nc.vector.tensor_copy, nc.scalar.copy]
        cp_idx = 0

        def copy_any(dst, src):
            nonlocal cp_idx
            copy_engines[cp_idx % 2](dst, src)
            cp_idx += 1

        # ---- phases 0 + 1, in two T halves to bound xT footprint ----
        with ExitStack() as p01:
            xa_pool = p01.enter_context(tc.tile_pool(name="xa", bufs=3))
            xT_pool = p01.enter_context(tc.tile_pool(name="xT", bufs=1))
            w_pool = p01.enter_context(tc.tile_pool(name="w", bufs=8))
            psT = p01.enter_context(tc.tile_pool(name="psT", bufs=2, space="PSUM"))
            psQK = p01.enter_context(tc.tile_pool(name="psQK", bufs=4, space="PSUM"))

            for th in range(2):
                TH = T // 2  # 1024 t per half
                xT = xT_pool.tile([128, NC_, TH], F32R, tag="xT")
                # phase 0: transpose this half of x
                for tt8 in range(8):
                    tt = th * 8 + tt8
                    xa = xa_pool.tile([128, C], F32R)
                    nc.sync.dma_start(xa[:], x_d[tt * 128:(tt + 1) * 128, :])
                    for cq in range(2):      # 4 transposes per psum bank
                        pt = psT.tile([128, 512], F32R)
                        for j in range(4):
                            cb = cq * 4 + j
                            nc.tensor.transpose(
                                pt[:, j * 128:(j + 1) * 128],
                                xa[:, cb * 128:(cb + 1) * 128], ident[:])
                        # strided copy into xT: [128, 4 c-planes, 128 t]
                        copy_any(
                            xT[:, cq * 4:(cq + 1) * 4, tt8 * 128:(tt8 + 1) * 128],
                            pt[:].rearrange("p (c t) -> p c t", c=4))

                # phase 1a: qT (padded layout) / kT for this half
                for wd, is_q in ((wq_d, True), (wk_d, False)):
                    w_tiles = []
                    for ct in range(NC_):
                        wt = w_pool.tile([128, GC], F32R, tag="w")
                        nc.sync.dma_start(wt[:], wd[ct * 128:(ct + 1) * 128, :])
                        w_tiles.append(wt)
                    for mt in range(4):
                        for nt in range(2):   # 512-wide t chunks in this half
                            pq = psQK.tile([128, 512], F32, tag="qkv")
                            for ct in range(NC_):
                                nc.tensor.matmul(
                                    pq[:],
                                    w_tiles[ct][:, mt * 128:(mt + 1) * 128],
                                    xT[:, ct, nt * 512:(nt + 1) * 512],
                                    start=(ct == 0), stop=(ct == NC_ - 1))
                            span = slice(th * TH + nt * 512, th * TH + (nt + 1) * 512)
                            if is_q:
                                # head 2mt -> partitions 0:64 of slot 2mt,
                                # head 2mt+1 -> partitions 64:128 of slot 2mt+1
                                copy_any(qT_pad[0:64, 2 * mt, span], pq[0:64, :])
                                copy_any(qT_pad[64:128, 2 * mt + 1, span], pq[64:128, :])
                            else:
                                copy_any(kT_sb[:, mt, span], pq[:])

                # phase 1b: v for this half (natural layout, into v_aug)
                wv_tiles = []
                for ct in range(NC_):
                    wt = w_pool.tile([128, GC], F32R, tag="w")
                    nc.sync.dma_start(wt[:], wv_d[ct * 128:(ct + 1) * 128, :])
                    wv_tiles.append(wt)
                for tt8 in range(8):
                    tt = th * 8 + tt8
                    pv = psQK.tile([128, 512], F32, tag="qkv")
                    for ct in range(NC_):
                        nc.tensor.matmul(
                            pv[:],
                            xT[:, ct, tt8 * 128:(tt8 + 1) * 128],
                            wv_tiles[ct][:],
                            start=(ct == 0), stop=(ct == NC_ - 1))
                    copy_any(
                        v_aug[:, :, tt, 0:64],
                        pv[:].rearrange("p (h d) -> p h d", h=H_PER_CORE))

        # preload the proj weights before attention so phase 3 starts hot
        wp_pool = ctx.enter_context(tc.tile_pool(name="wp", bufs=1))
        wp_sb = wp_pool.tile([128, 4, C], F32R)
        for kt in range(4):
            nc.sync.dma_start(wp_sb[:, kt, :], wp_d[kt * 128:(kt + 1) * 128, :])

        # ---- phase 2: attention ----
        with ExitStack() as p2:
            pT_pool = p2.enter_context(tc.tile_pool(name="pT", bufs=6))
            ySt_pool = p2.enter_context(tc.tile_pool(name="ySt", bufs=6))
            rc_pool = p2.enter_context(tc.tile_pool(name="rc", bufs=3))
            rb_pool = p2.enter_context(tc.tile_pool(name="rb", bufs=2))
            psS = p2.enter_context(tc.tile_pool(name="psS", bufs=3, space="PSUM"))
            psY = p2.enter_context(tc.tile_pool(name="psY", bufs=2, space="PSUM"))

            for h in range(H_PER_CORE):
                mt_h = h // 2
                for qsb in range(NQSB):
                    nkb = 4 * (qsb + 1)
                    yTp = psY.tile([65, 512], F32, tag="yTp")
                    # Full-width prefix blocks first (pairs sharing one
                    # 2-bank psum window and one batched exp), then the 4
                    # diagonal blocks, width-trimmed, paired two per window.
                    prefix = [(kb, 0) for kb in range(4 * qsb)]
                    diag = [(kb, kb * 128 - qsb * 512)
                            for kb in range(4 * qsb, nkb)]
                    groups = [prefix[i:i + 2] for i in range(0, len(prefix), 2)]
                    groups += [diag[0:2], diag[2:4]]
                    n_pv = 0
                    for g in groups:
                        wide = psS.tile([128, 1024], F32, tag="wide")
                        pTw = pT_pool.tile([128, 1024], F16, tag="pTw")
                        diag_group = g[0][0] >= 4 * qsb
                        for i, (kb, lo) in enumerate(g):
                            nc.tensor.matmul(
                                wide[:, i * 512 + lo:(i + 1) * 512],
                                kT_sb[:, mt_h, kb * 128:(kb + 1) * 128],
                                qT_pad[:, h, qsb * 512 + lo:(qsb + 1) * 512],
                                start=True, stop=True)
                        if diag_group:
                            # per-block exp, width-trimmed to the causal range
                            for i, (kb, lo) in enumerate(g):
                                nc.scalar.activation(
                                    pTw[:, i * 512 + lo:(i + 1) * 512],
                                    wide[:, i * 512 + lo:(i + 1) * 512],
                                    AF.Exp, bias=0.0, scale=SCALE)
                        else:
                            nc.scalar.activation(
                                pTw[:, 0:len(g) * 512], wide[:, 0:len(g) * 512],
                                AF.Exp, bias=0.0, scale=SCALE)
                        for i, (kb, lo) in enumerate(g):
                            if diag_group:
                                # zero the strictly-upper triangle
                                nc.vector.tensor_mul(
                                    pTw[:, i * 512 + lo:i * 512 + lo + 128],
                                    pTw[:, i * 512 + lo:i * 512 + lo + 128],
                                    tri_mask[:])
                            nc.tensor.matmul(
                                yTp[:, lo:512],
                                v_aug[:, h, kb, :],
                                pTw[:, i * 512 + lo:(i + 1) * 512],
                                start=(n_pv == 0), stop=(n_pv == nkb - 1))
                            n_pv += 1
                    # stage yT_aug out of PSUM immediately (frees the psum
                    # slot); the slow 1-lane reciprocal chain then runs off
                    # the critical path entirely in SBUF.
                    ySt = ySt_pool.tile([65, 512], F32, tag="ySt")
                    nc.vector.tensor_copy(ySt[:], yTp[:])
                    # denominator row to partition 0: the custom-DVE fast
                    # reciprocal mishandles partition-offset inputs
                    dn = rc_pool.tile([1, 512], F32, tag="dn")
                    nc.scalar.copy(dn[:], yTp[64:65, :])
                    recip = rc_pool.tile([1, 512], F32, tag="recip")
                    nc.vector.reciprocal_approx_fast(recip[:], dn[:])
                    rbc = rb_pool.tile([64, 512], F32)
                    nc.gpsimd.partition_broadcast(rbc[:], recip[:])
                    nc.vector.tensor_mul(
                        yT_sb[64 * (h % 2):64 * (h % 2) + 64, mt_h,
                              qsb * 512:(qsb + 1) * 512],
                        ySt[0:64, :], rbc[:])

            # ---- phase 3: output projection (borrows psS wide slots,
            # so no psum pool transition / drain before it starts) ----
            so_pool = p2.enter_context(tc.tile_pool(name="so", bufs=3))
            for mt in range(NT):
                for n2 in range(2):
                    wide = psS.tile([128, 1024], F32, tag="wide")
                    for kt in range(4):
                        nc.tensor.matmul(
                            wide[:, 0:512],
                            yT_sb[:, kt, mt * 128:(mt + 1) * 128],
                            wp_sb[:, kt, n2 * 512:(n2 + 1) * 512],
                            start=(kt == 0), stop=(kt == 3))
                    so = so_pool.tile([128, 512], F32)
                    copy_any(so[:], wide[:, 0:512])
                    nc.sync.dma_start(
                        out_d[mt * 128:(mt + 1) * 128, n2 * 512:(n2 + 1) * 512],
                        so[:])

_CACHE = {}


def _get_nc():
    if "nc" not in _CACHE:
        nc = bacc.Bacc("TRN2", target_bir_lowering=False, debug=False,
                       num_devices=N_CORES)
        build(nc)
        nc.compile()
        _CACHE["nc"] = nc
    return _CACHE["nc"]


def make_in_maps(x, w_attn, w_proj):
    x = np.asarray(x, dtype=np.float32)
    w_attn = np.asarray(w_attn, dtype=np.float32)
    w_proj = np.asarray(w_proj, dtype=np.float32)
    in_maps = []
    for core in range(N_CORES):
        b, hg = divmod(core, 2)
        cs = slice(hg * GC, (hg + 1) * GC)
        in_maps.append({
            "x": np.ascontiguousarray(x[b]),
            "wq": np.ascontiguousarray(w_attn[:, 0 * C:1 * C][:, cs]),
            "wk": np.ascontiguousarray(w_attn[:, 1 * C:2 * C][:, cs]),
            "wv": np.ascontiguousarray(w_attn[:, 2 * C:3 * C][:, cs]),
            "wp": np.ascontiguousarray(w_proj[cs, :]),
        })
    return in_maps


def kernel(x, w_attn, w_proj, _trace=False, _trace_kwargs=None):
    nc = _get_nc()
    in_maps = make_in_maps(x, w_attn, w_proj)
    res = None
    for attempt in range(3):
        try:
            res = run_bass_kernel_spmd(nc, in_maps,
                                       core_ids=list(range(N_CORES)),
                                       trace=_trace, **(_trace_kwargs or {}))
            break
        except Exception:
            # a previous process can leave the device wedged
            # (NRT_EXEC_UNIT_UNRECOVERABLE); a retry recovers it
            if attempt == 2:
                raise
    _CACHE["last_results"] = res
    B = np.asarray(x).shape[0]
    out = np.empty((B, T, C), dtype=np.float32)
    for b in range(B):
        out[b] = res.results[2 * b]["out"] + res.results[2 * b + 1]["out"]
    return out



# revision 3
# speedup vs baseline: 1.3344x; 1.3344x over previous
"""Causal self-attention (B=4, T=2048, C=1024, H=16) on 8 TRN2 NeuronCores.

Sharding: core = (batch b, head-group hg). Data parallel over B (4), tensor
parallel over heads (2 groups of 8). Each core computes a partial output
projection for its 8 heads; the host sums the two partials per batch.

v2 design (vs baseline):
  - host passes xT (pre-transposed) and all inputs in bf16: no PE
    transposes, FWL weight loads, half the DMA bytes.
  - q/k stored pair-packed [128, 4 pairs, T]: head 2mt on partitions 0:64,
    head 2mt+1 on 64:128. Scores run as row-tiled K=64 matmul PAIRS
    (tile_position (0,0)/(64,0) auto-derived from base partitions), two
    heads concurrently per 512-cycle stream -> no zero-pad waste.
  - single flat region, nt-chunked emission (qkv(nt) -> attention(qsb=nt)
    -> proj(nt)) so qkv/attention/proj pipeline across engines.
  - ACT does exp only (2D-AP trimmed diag exps); all PSUM evacuations on
    DVE; tri-mask + partition broadcast on GpSimd.
  - PSUM: psS 3x[128,1024] (scores/qkv/v/proj wides) + psY 2x[65,512]
    (yT accumulators) = exactly 8 banks.
"""
import numpy as np
import ml_dtypes
from contextlib import ExitStack

import concourse.bass as bass
import concourse.mybir as mybir
import concourse.tile as tile
from concourse import bacc
from concourse.bass_utils import run_bass_kernel_spmd

F32 = mybir.dt.float32
BF16 = mybir.dt.bfloat16
F16 = mybir.dt.float16
AF = mybir.ActivationFunctionType

T = 2048
C = 1024
H_PER_CORE = 8          # heads per core
D = 64                  # head dim
GC = H_PER_CORE * D     # 512 channels per head-group
SCALE = 1.0 / 32.0      # C ** -0.5
N_CORES = 8
NT = T // 128           # 16 t-tiles
NC_ = C // 128          # 8 c-tiles
NQSB = T // 512         # 4 q superblocks


def build(nc):
    xT_d = nc.dram_tensor("xT", [C, T], BF16, kind="ExternalInput").ap()
    wq_d = nc.dram_tensor("wq", [C, GC], BF16, kind="ExternalInput").ap()
    wk_d = nc.dram_tensor("wk", [C, GC], BF16, kind="ExternalInput").ap()
    wv_d = nc.dram_tensor("wv", [C, GC], BF16, kind="ExternalInput").ap()
    wp_d = nc.dram_tensor("wp", [GC, C], BF16, kind="ExternalInput").ap()
    out_d = nc.dram_tensor("out", [T, C], BF16, kind="ExternalOutput").ap()

    with tile.TileContext(nc) as tc, ExitStack() as ctx:
        const = ctx.enter_context(tc.tile_pool(name="const", bufs=1))
        persist = ctx.enter_context(tc.tile_pool(name="persist", bufs=1))
        pT_pool = ctx.enter_context(tc.tile_pool(name="pT", bufs=6))
        ySt_pool = ctx.enter_context(tc.tile_pool(name="ySt", bufs=4))
        rc_pool = ctx.enter_context(tc.tile_pool(name="rc", bufs=4))
        rb_pool = ctx.enter_context(tc.tile_pool(name="rb", bufs=3))
        so_pool = ctx.enter_context(tc.tile_pool(name="so", bufs=3))
        psS = ctx.enter_context(tc.tile_pool(name="psS", bufs=3, space="PSUM"))
        psY = ctx.enter_context(tc.tile_pool(name="psY", bufs=2, space="PSUM"))

        # tri_mask[k, j] = 1.0 if k <= j else 0.0 (zero strictly-lower => keep
        # k<=j upper... matches baseline: zero the strictly-upper triangle of
        # scoresT where k > q)
        tri_mask = const.tile([128, 128], F16)
        nc.gpsimd.memset(tri_mask[:], 1.0)
        nc.gpsimd.affine_select(
            out=tri_mask[:], in_=tri_mask[:],
            compare_op=mybir.AluOpType.is_ge, fill=0.0, base=0,
            pattern=[[1, 128]], channel_multiplier=-1,
        )

        # persistent SBUF
        xT_sb = persist.tile([128, NC_, T], BF16)
        qT_sb = persist.tile([128, 4, T], BF16)
        kT_sb = persist.tile([128, 4, T], BF16)
        v_aug = persist.tile([128, H_PER_CORE, NT, 65], F16)
        yT_sb = persist.tile([128, 4, T], BF16)
        wq_sb = persist.tile([128, NC_, GC], BF16)
        wk_sb = persist.tile([128, NC_, GC], BF16)
        wv_sb = persist.tile([128, NC_, GC], BF16)
        wp_sb = persist.tile([128, 4, C], BF16)
        nc.gpsimd.memset(v_aug[:, :, :, 64], 1.0)

        # ---- DMAs: weights first, then xT chunks in nt order ----
        for ct in range(NC_):
            nc.sync.dma_start(wq_sb[:, ct, :], wq_d[ct * 128:(ct + 1) * 128, :])
        for ct in range(NC_):
            nc.sync.dma_start(
                xT_sb[:, ct, 0:512], xT_d[ct * 128:(ct + 1) * 128, 0:512])
        for ct in range(NC_):
            nc.sync.dma_start(wk_sb[:, ct, :], wk_d[ct * 128:(ct + 1) * 128, :])
        for ct in range(NC_):
            nc.sync.dma_start(wv_sb[:, ct, :], wv_d[ct * 128:(ct + 1) * 128, :])
        for kt in range(4):
            nc.sync.dma_start(wp_sb[:, kt, :], wp_d[kt * 128:(kt + 1) * 128, :])
        for ntd in range(1, NQSB):
            for ct in range(NC_):
                nc.sync.dma_start(
                    xT_sb[:, ct, ntd * 512:(ntd + 1) * 512],
                    xT_d[ct * 128:(ct + 1) * 128, ntd * 512:(ntd + 1) * 512])

        for nt in range(NQSB):
            nsp = slice(nt * 512, (nt + 1) * 512)
            # ---- qkv for this t-chunk ----
            for mt in range(4):
                wide = psS.tile([128, 1024], F32, tag="wide")
                msp = slice(mt * 128, (mt + 1) * 128)
                for ct in range(NC_):
                    nc.tensor.matmul(
                        wide[:, 0:512], wq_sb[:, ct, msp], xT_sb[:, ct, nsp],
                        start=(ct == 0), stop=(ct == NC_ - 1))
                for ct in range(NC_):
                    nc.tensor.matmul(
                        wide[:, 512:1024], wk_sb[:, ct, msp], xT_sb[:, ct, nsp],
                        start=(ct == 0), stop=(ct == NC_ - 1))
                nc.vector.tensor_copy(qT_sb[:, mt, nsp], wide[:, 0:512])
                nc.vector.tensor_copy(kT_sb[:, mt, nsp], wide[:, 512:1024])
            for tp in range(2):
                wide = psS.tile([128, 1024], F32, tag="wide")
                for i in range(2):
                    tt = 4 * nt + 2 * tp + i
                    for ct in range(NC_):
                        nc.tensor.matmul(
                            wide[:, i * 512:(i + 1) * 512],
                            xT_sb[:, ct, tt * 128:(tt + 1) * 128],
                            wv_sb[:, ct, :],
                            start=(ct == 0), stop=(ct == NC_ - 1))
                    nc.vector.tensor_copy(
                        v_aug[:, :, tt, 0:64],
                        wide[:, i * 512:(i + 1) * 512].rearrange(
                            "p (h d) -> p h d", h=H_PER_CORE))

            # ---- attention for qsb = nt ----
            nkb = 4 * (nt + 1)
            for mt in range(4):
                yTe = psY.tile([65, 512], F32, tag="yT")
                yTo = psY.tile([65, 512], F32, tag="yT")
                for kb in range(nkb):
                    lo = max(0, kb * 128 - nt * 512)
                    diag = kb >= 4 * nt
                    ksp = slice(kb * 128, (kb + 1) * 128)
                    qsl = slice(nt * 512 + lo, (nt + 1) * 512)
                    wide = psS.tile([128, 1024], F32, tag="wide")
                    nc.tensor.matmul(
                        wide[:, lo:512],
                        kT_sb[0:64, mt, ksp], qT_sb[0:64, mt, qsl],
                        start=True, stop=True)
                    nc.tensor.matmul(
                        wide[:, 512 + lo:1024],
                        kT_sb[64:128, mt, ksp], qT_sb[64:128, mt, qsl],
                        start=True, stop=True)
                    pT = pT_pool.tile([128, 1024], F16, tag="pT")
                    if lo == 0:
                        nc.scalar.activation(
                            pT[:, 0:1024], wide[:, 0:1024],
                            AF.Exp, bias=0.0, scale=SCALE)
                    else:
                        w2 = wide[:].rearrange("p (h q) -> p h q", h=2)
                        p2 = pT[:].rearrange("p (h q) -> p h q", h=2)
                        nc.scalar.activation(
                            p2[:, :, lo:512], w2[:, :, lo:512],
                            AF.Exp, bias=0.0, scale=SCALE)
                    if diag:
                        nc.vector.tensor_mul(
                            pT[:, lo:lo + 128], pT[:, lo:lo + 128], tri_mask[:])
                        nc.vector.tensor_mul(
                            pT[:, 512 + lo:512 + lo + 128],
                            pT[:, 512 + lo:512 + lo + 128], tri_mask[:])
                    nc.tensor.matmul(
                        yTe[:, lo:512], v_aug[:, 2 * mt, kb, :], pT[:, lo:512],
                        start=(kb == 0), stop=(kb == nkb - 1))
                    nc.tensor.matmul(
                        yTo[:, lo:512], v_aug[:, 2 * mt + 1, kb, :],
                        pT[:, 512 + lo:1024],
                        start=(kb == 0), stop=(kb == nkb - 1))
                for par, yTp in ((0, yTe), (1, yTo)):
                    # stage out of PSUM immediately to free the bank; the
                    # slow reciprocal chain runs off the critical path
                    ySt = ySt_pool.tile([65, 512], F32, tag="ySt")
                    nc.vector.tensor_copy(ySt[:], yTp[:])
                    # denominator to partition 0 (fast recip needs base 0)
                    dn = rc_pool.tile([1, 512], F32, tag="dn")
                    nc.scalar.copy(dn[:], yTp[64:65, :])
                    recip = rc_pool.tile([1, 512], F32, tag="recip")
                    nc.vector.reciprocal_approx_fast(recip[:], dn[:])
                    rbc = rb_pool.tile([64, 512], F32)
                    nc.gpsimd.partition_broadcast(rbc[:], recip[:])
                    nc.vector.tensor_mul(
                        yT_sb[64 * par:64 * par + 64, mt, nsp],
                        ySt[0:64, :], rbc[:])

            # ---- output projection for this t-chunk ----
            for tt in range(4 * nt, 4 * nt + 4):
                wide = psS.tile([128, 1024], F32, tag="wide")
                tsp = slice(tt * 128, (tt + 1) * 128)
                for n2 in range(2):
                    for kt in range(4):
                        nc.tensor.matmul(
                            wide[:, n2 * 512:(n2 + 1) * 512],
                            yT_sb[:, kt, tsp],
                            wp_sb[:, kt, n2 * 512:(n2 + 1) * 512],
                            start=(kt == 0), stop=(kt == 3))
                so = so_pool.tile([128, 1024], BF16)
                nc.vector.tensor_copy(so[:], wide[:])
                nc.sync.dma_start(out_d[tsp, :], so[:])


_CACHE = {}


def _get_nc():
    if "nc" not in _CACHE:
        nc = bacc.Bacc("TRN2", target_bir_lowering=False, debug=False,
                       num_devices=N_CORES)
        build(nc)
        nc.compile()
        _CACHE["nc"] = nc
    return _CACHE["nc"]


def make_in_maps(x, w_attn, w_proj):
    x = np.asarray(x, dtype=np.float32)
    w_attn = np.asarray(w_attn, dtype=np.float32)
    w_proj = np.asarray(w_proj, dtype=np.float32)
    bf = ml_dtypes.bfloat16
    in_maps = []
    for core in range(N_CORES):
        b, hg = divmod(core, 2)
        cs = slice(hg * GC, (hg + 1) * GC)
        in_maps.append({
            "xT": np.ascontiguousarray(x[b].T).astype(bf),
            "wq": np.ascontiguousarray(w_attn[:, 0 * C:1 * C][:, cs]).astype(bf),
            "wk": np.ascontiguousarray(w_attn[:, 1 * C:2 * C][:, cs]).astype(bf),
            "wv": np.ascontiguousarray(w_attn[:, 2 * C:3 * C][:, cs]).astype(bf),
            "wp": np.ascontiguousarray(w_proj[cs, :]).astype(bf),
        })
    return in_maps


def kernel(x, w_attn, w_proj, _trace=False, _trace_kwargs=None):
    nc = _get_nc()
    in_maps = make_in_maps(x, w_attn, w_proj)
    res = None
    for attempt in range(3):
        try:
            res = run_bass_kernel_spmd(nc, in_maps,
                                       core_ids=list(range(N_CORES)),
                                       trace=_trace, **(_trace_kwargs or {}))
            break
        except Exception:
            # a previous process can leave the device wedged
            # (NRT_EXEC_UNIT_UNRECOVERABLE); a retry recovers it
            if attempt == 2:
                raise
    _CACHE["last_results"] = res
    B = np.asarray(x).shape[0]
    out = np.empty((B, T, C), dtype=np.float32)
    for b in range(B):
        out[b] = (res.results[2 * b]["out"].astype(np.float32)
                  + res.results[2 * b + 1]["out"].astype(np.float32))
    return out


# revision 5
# speedup vs baseline: 1.4451x; 1.0830x over previous
"""Causal self-attention (B=4, T=2048, C=1024, H=16) on 8 TRN2 NeuronCores.

Sharding: core = (batch b, head-group hg). Data parallel over B (4), tensor
parallel over heads (2 groups of 8). Each core computes a partial output
projection for its 8 heads; the host sums the two partials per batch.

v3 design:
  - host passes xT (pre-transposed) and all inputs in bf16: no PE
    transposes, FWL weight loads, half the DMA bytes.
  - q/k stored pair-packed [128, 4 pairs, T]: head 2mt on partitions 0:64,
    head 2mt+1 on 64:128. Scores run as row-tiled K=64 matmul PAIRS
    (tile_position (0,0)/(64,0) auto-derived from base partitions), two
    heads concurrently per 512-cycle stream -> no zero-pad waste.
  - the attention inner loop is ACT(exp)-bound (1147ns exp vs 640ns of PE
    work per k-block) and the PE queue is in-order, so qkv(nt+1) and
    proj(nt-2..) matmul chains are INTERLEAVED into attention(nt)'s
    emission as filler units to keep the PE busy while ACT grinds exps.
  - 32 warmup matmuls at t=0 so the PE HAM clock-gate opens (1.2->2.4GHz)
    before the real qkv work arrives (saves ~20us of cold penalty).
  - ACT does exp (2D-AP trimmed diag exps) + tiny dn copies; all PSUM
    evacuations on DVE; partition broadcast on GpSimd.
  - PSUM: psS 3x[128,1024] (scores/qkv/v/proj wides) + psY 2x[65,512]
    (yT accumulators) = exactly 8 banks.
"""
import numpy as np
import ml_dtypes
from contextlib import ExitStack

import concourse.bass as bass
import concourse.mybir as mybir
import concourse.tile as tile
from concourse import bacc
from concourse.bass_utils import run_bass_kernel_spmd

F32 = mybir.dt.float32
BF16 = mybir.dt.bfloat16
F16 = mybir.dt.float16
AF = mybir.ActivationFunctionType

T = 2048
C = 1024
H_PER_CORE = 8          # heads per core
D = 64                  # head dim
GC = H_PER_CORE * D     # 512 channels per head-group
SCALE = 1.0 / 32.0      # C ** -0.5
N_CORES = 8
NT = T // 128           # 16 t-tiles
NC_ = C // 128          # 8 c-tiles
NQSB = T // 512         # 4 q superblocks


def build(nc):
    xT_d = nc.dram_tensor("xT", [C, T], BF16, kind="ExternalInput").ap()
    wq_d = nc.dram_tensor("wq", [C, GC], BF16, kind="ExternalInput").ap()
    wk_d = nc.dram_tensor("wk", [C, GC], BF16, kind="ExternalInput").ap()
    wv_d = nc.dram_tensor("wv", [C, GC], BF16, kind="ExternalInput").ap()
    wp_d = nc.dram_tensor("wp", [GC, C], BF16, kind="ExternalInput").ap()
    out_d = nc.dram_tensor("out", [T, C], BF16, kind="ExternalOutput").ap()

    with tile.TileContext(nc) as tc, ExitStack() as ctx:
        const = ctx.enter_context(tc.tile_pool(name="const", bufs=1))
        persist = ctx.enter_context(tc.tile_pool(name="persist", bufs=1))
        pT_pool = ctx.enter_context(tc.tile_pool(name="pT", bufs=6))
        ySt_pool = ctx.enter_context(tc.tile_pool(name="ySt", bufs=4))
        rc_pool = ctx.enter_context(tc.tile_pool(name="rc", bufs=4))
        rb_pool = ctx.enter_context(tc.tile_pool(name="rb", bufs=3))
        so_pool = ctx.enter_context(tc.tile_pool(name="so", bufs=3))
        psS = ctx.enter_context(tc.tile_pool(name="psS", bufs=3, space="PSUM"))
        psY = ctx.enter_context(tc.tile_pool(name="psY", bufs=2, space="PSUM"))

        # tri_mask[k, j] = 1.0 if k <= j else 0.0
        tri_mask = const.tile([128, 128], F16)
        nc.gpsimd.memset(tri_mask[:], 1.0)
        nc.gpsimd.affine_select(
            out=tri_mask[:], in_=tri_mask[:],
            compare_op=mybir.AluOpType.is_ge, fill=0.0, base=0,
            pattern=[[1, 128]], channel_multiplier=-1,
        )

        # persistent SBUF
        xT_sb = persist.tile([128, NC_, T], BF16)
        qT_sb = persist.tile([128, 4, T], BF16)
        kT_sb = persist.tile([128, 4, T], BF16)
        v_aug = persist.tile([128, H_PER_CORE, NT, 65], F16)
        yT_sb = persist.tile([128, 4, T], BF16)
        wq_sb = persist.tile([128, NC_, GC], BF16)
        wk_sb = persist.tile([128, NC_, GC], BF16)
        wv_sb = persist.tile([128, NC_, GC], BF16)
        wp_sb = persist.tile([128, 4, C], BF16)
        nc.gpsimd.memset(v_aug[:, :, :, 64], 1.0)

        # ---- PE warmup: ~3.4us of back-to-back matmuls so the HAM clock
        # gate opens before the first real qkv matmul arrives ----
        warm = psS.tile([128, 1024], F32, tag="wide")
        for i in range(32):
            nc.tensor.matmul(warm[:, 0:128], tri_mask[:], tri_mask[:],
                             start=True, stop=True)

        # ---- DMAs: weights first, then xT chunks in nt order ----
        for ct in range(NC_):
            nc.sync.dma_start(wq_sb[:, ct, :], wq_d[ct * 128:(ct + 1) * 128, :])
        for ct in range(NC_):
            nc.sync.dma_start(
                xT_sb[:, ct, 0:512], xT_d[ct * 128:(ct + 1) * 128, 0:512])
        for ct in range(NC_):
            nc.sync.dma_start(wk_sb[:, ct, :], wk_d[ct * 128:(ct + 1) * 128, :])
        for ct in range(NC_):
            nc.sync.dma_start(wv_sb[:, ct, :], wv_d[ct * 128:(ct + 1) * 128, :])
        for kt in range(4):
            nc.sync.dma_start(wp_sb[:, kt, :], wp_d[kt * 128:(kt + 1) * 128, :])
        for ntd in range(1, NQSB):
            for ct in range(NC_):
                nc.sync.dma_start(
                    xT_sb[:, ct, ntd * 512:(ntd + 1) * 512],
                    xT_d[ct * 128:(ct + 1) * 128, ntd * 512:(ntd + 1) * 512])

        # ---- emit helpers (each returns a generator-style closure) ----
        def emit_qk(nt, mt):
            nsp = slice(nt * 512, (nt + 1) * 512)
            msp = slice(mt * 128, (mt + 1) * 128)
            wide = psS.tile([128, 1024], F32, tag="wide")
            for ct in range(NC_):
                nc.tensor.matmul(
                    wide[:, 0:512], wq_sb[:, ct, msp], xT_sb[:, ct, nsp],
                    start=(ct == 0), stop=(ct == NC_ - 1))
            for ct in range(NC_):
                nc.tensor.matmul(
                    wide[:, 512:1024], wk_sb[:, ct, msp], xT_sb[:, ct, nsp],
                    start=(ct == 0), stop=(ct == NC_ - 1))
            nc.vector.tensor_copy(qT_sb[:, mt, nsp], wide[:, 0:512])
            nc.vector.tensor_copy(kT_sb[:, mt, nsp], wide[:, 512:1024])

        def emit_v(nt, tp):
            wide = psS.tile([128, 1024], F32, tag="wide")
            for i in range(2):
                tt = 4 * nt + 2 * tp + i
                for ct in range(NC_):
                    nc.tensor.matmul(
                        wide[:, i * 512:(i + 1) * 512],
                        xT_sb[:, ct, tt * 128:(tt + 1) * 128],
                        wv_sb[:, ct, :],
                        start=(ct == 0), stop=(ct == NC_ - 1))
                nc.vector.tensor_copy(
                    v_aug[:, :, tt, 0:64],
                    wide[:, i * 512:(i + 1) * 512].rearrange(
                        "p (h d) -> p h d", h=H_PER_CORE))

        def emit_proj(tt):
            wide = psS.tile([128, 1024], F32, tag="wide")
            tsp = slice(tt * 128, (tt + 1) * 128)
            for n2 in range(2):
                for kt in range(4):
                    nc.tensor.matmul(
                        wide[:, n2 * 512:(n2 + 1) * 512],
                        yT_sb[:, kt, tsp],
                        wp_sb[:, kt, n2 * 512:(n2 + 1) * 512],
                        start=(kt == 0), stop=(kt == 3))
            so = so_pool.tile([128, 1024], BF16)
            for h in range(2):
                hs = slice(h * 512, (h + 1) * 512)
                nc.vector.tensor_copy(so[:, hs], wide[:, hs])
                nc.sync.dma_start(out_d[tsp, hs], so[:, hs])

        def emit_att_step(nt, mt, kb, nkb, yTe, yTo):
            lo = max(0, kb * 128 - nt * 512)
            diag = kb >= 4 * nt
            ksp = slice(kb * 128, (kb + 1) * 128)
            qsl = slice(nt * 512 + lo, (nt + 1) * 512)
            wide = psS.tile([128, 1024], F32, tag="wide")
            nc.tensor.matmul(
                wide[:, lo:512],
                kT_sb[0:64, mt, ksp], qT_sb[0:64, mt, qsl],
                start=True, stop=True)
            nc.tensor.matmul(
                wide[:, 512 + lo:1024],
                kT_sb[64:128, mt, ksp], qT_sb[64:128, mt, qsl],
                start=True, stop=True)
            pT = pT_pool.tile([128, 1024], F16, tag="pT")
            if lo == 0:
                nc.scalar.activation(
                    pT[:, 0:1024], wide[:, 0:1024],
                    AF.Exp, bias=0.0, scale=SCALE)
            else:
                w2 = wide[:].rearrange("p (h q) -> p h q", h=2)
                p2 = pT[:].rearrange("p (h q) -> p h q", h=2)
                nc.scalar.activation(
                    p2[:, :, lo:512], w2[:, :, lo:512],
                    AF.Exp, bias=0.0, scale=SCALE)
            if diag:
                nc.vector.tensor_mul(
                    pT[:, lo:lo + 128], pT[:, lo:lo + 128], tri_mask[:])
                nc.vector.tensor_mul(
                    pT[:, 512 + lo:512 + lo + 128],
                    pT[:, 512 + lo:512 + lo + 128], tri_mask[:])
            nc.tensor.matmul(
                yTe[:, lo:512], v_aug[:, 2 * mt, kb, :], pT[:, lo:512],
                start=(kb == 0), stop=(kb == nkb - 1))
            nc.tensor.matmul(
                yTo[:, lo:512], v_aug[:, 2 * mt + 1, kb, :],
                pT[:, 512 + lo:1024],
                start=(kb == 0), stop=(kb == nkb - 1))

        def emit_norm(nt, mt, par, yTp):
            nsp = slice(nt * 512, (nt + 1) * 512)
            # stage out of PSUM immediately to free the bank; the slow
            # reciprocal chain runs off the critical path
            ySt = ySt_pool.tile([65, 512], F32, tag="ySt")
            nc.vector.tensor_copy(ySt[:], yTp[:])
            dn = rc_pool.tile([1, 512], F32, tag="dn")
            nc.scalar.copy(dn[:], yTp[64:65, :])
            recip = rc_pool.tile([1, 512], F32, tag="recip")
            nc.vector.reciprocal_approx_fast(recip[:], dn[:])
            rbc = rb_pool.tile([64, 512], F32)
            nc.gpsimd.partition_broadcast(rbc[:], recip[:])
            nc.vector.tensor_mul(
                yT_sb[64 * par:64 * par + 64, mt, nsp],
                ySt[0:64, :], rbc[:])

        # ---- main emission: qkv(0) prologue, then per-nt attention with
        # qkv(nt+1) and pending proj chains interleaved as PE filler ----
        for mt in range(4):
            emit_qk(0, mt)
        for tp in range(2):
            emit_v(0, tp)

        pending_proj = []        # tt indices whose proj is ready to emit
        for nt in range(NQSB):
            nkb = 4 * (nt + 1)
            # filler units: qkv chains for nt+1, plus any pending proj
            fillers = []
            if nt + 1 < NQSB:
                for mt in range(4):
                    fillers.append(("qk", nt + 1, mt))
                for tp in range(2):
                    fillers.append(("v", nt + 1, tp))
            for tt in pending_proj:
                fillers.append(("proj", tt, None))
            pending_proj = []

            steps = [(mt, kb) for mt in range(4) for kb in range(nkb)]
            n_steps = len(steps)
            nf = len(fillers)
            fill_at = {}
            for j in range(nf):
                fill_at.setdefault((j * n_steps) // nf, []).append(fillers[j])

            yT_pair = {}
            cur_mt = -1
            for si, (mt, kb) in enumerate(steps):
                if mt != cur_mt:
                    yTe = psY.tile([65, 512], F32, tag="yT")
                    yTo = psY.tile([65, 512], F32, tag="yT")
                    yT_pair[mt] = (yTe, yTo)
                    cur_mt = mt
                emit_att_step(nt, mt, kb, nkb, *yT_pair[mt])
                for f in fill_at.get(si, []):
                    if f[0] == "qk":
                        emit_qk(f[1], f[2])
                    elif f[0] == "v":
                        emit_v(f[1], f[2])
                    else:
                        emit_proj(f[1])
                if kb == nkb - 1:
                    yTe, yTo = yT_pair[mt]
                    emit_norm(nt, mt, 0, yTe)
                    emit_norm(nt, mt, 1, yTo)
            pending_proj = list(range(4 * nt, 4 * nt + 4))

        # last chunk's proj has no later attention window to hide in
        for tt in pending_proj:
            emit_proj(tt)


_CACHE = {}


def _get_nc():
    if "nc" not in _CACHE:
        nc = bacc.Bacc("TRN2", target_bir_lowering=False, debug=False,
                       num_devices=N_CORES)
        build(nc)
        nc.compile()
        _CACHE["nc"] = nc
    return _CACHE["nc"]


def make_in_maps(x, w_attn, w_proj):
    x = np.asarray(x, dtype=np.float32)
    w_attn = np.asarray(w_attn, dtype=np.float32)
    w_proj = np.asarray(w_proj, dtype=np.float32)
    bf = ml_dtypes.bfloat16
    in_maps = []
    for core in range(N_CORES):
        b, hg = divmod(core, 2)
        cs = slice(hg * GC, (hg + 1) * GC)
        in_maps.append({
            "xT": np.ascontiguousarray(x[b].T).astype(bf),
            "wq": np.ascontiguousarray(w_attn[:, 0 * C:1 * C][:, cs]).astype(bf),
            "wk": np.ascontiguousarray(w_attn[:, 1 * C:2 * C][:, cs]).astype(bf),
            "wv": np.ascontiguousarray(w_attn[:, 2 * C:3 * C][:, cs]).astype(bf),
            "wp": np.ascontiguousarray(w_proj[cs, :]).astype(bf),
        })
    return in_maps


def kernel(x, w_attn, w_proj, _trace=False, _trace_kwargs=None):
    nc = _get_nc()
    in_maps = make_in_maps(x, w_attn, w_proj)
    res = None
    for attempt in range(3):
        try:
            res = run_bass_kernel_spmd(nc, in_maps,
                                       core_ids=list(range(N_CORES)),
                                       trace=_trace, **(_trace_kwargs or {}))
            break
        except Exception:
            # a previous process can leave the device wedged
            # (NRT_EXEC_UNIT_UNRECOVERABLE); a retry recovers it
            if attempt == 2:
                raise
    _CACHE["last_results"] = res
    B = np.asarray(x).shape[0]
    out = np.empty((B, T, C), dtype=np.float32)
    for b in range(B):
        out[b] = (res.results[2 * b]["out"].astype(np.float32)
                  + res.results[2 * b + 1]["out"].astype(np.float32))
    return out
